# revision 1
# baseline (speedup 1.0000x reference)
"""DLRM (multi-table EmbeddingBag + MLPs) on 8 Trainium2 NeuronCores.

Strategy: data-parallel over batch (512 bags/core); embedding tables
replicated in each core's HBM as 104 window tensors (4 windows x 25000 rows
per table, so local row ids fit dma_gather's int16 index limit). The host
only reorders/pads index metadata; all table rows are fetched on-device via
dma_gather (4 SWDGE queues). Pooling uses a per-128-slot-group selection
matmul (gathered rows as stationary lhsT, on-chip one-hot bag matrix as
moving rhs) accumulating into PSUM - the same duplicate-safe reduction
pattern as tile_scatter_add. Bottom/top MLPs run fused in the same kernel,
feature-major, in fp32.
"""
import numpy as np

import concourse.bacc as bacc
import concourse.bass as bass
import concourse.mybir as mybir
import concourse.tile as tile
from concourse.bass_utils import run_bass_kernel_spmd

T = 26          # tables
R = 100000      # rows per table
E = 64          # embedding dim
B = 4096        # batch
L = 32          # lookups per bag
BOT = [256, 512, 256, 64]
TOP = [E * (1 + T), 512, 256, 1]   # 1728 -> 512 -> 256 -> 1
N_CORES = 8
B_CORE = B // N_CORES               # 512 bags per core
N_BT = B_CORE // 128                # 4 bag-tiles per core
W_ROWS = 25000                      # window rows (<= int16 range)
N_WIN = R // W_ROWS                 # 4 windows per table
P = 128
ZF = TOP[0] + 64                    # 1792: zero-padded feature dim
NZCH = ZF // P                      # 14 z chunks


def _pack_idx_block(idx_i16):
    """[n] int16 (n % 16 == 0) -> [128, n//16]: j -> (j%16, j//16), replicated x8."""
    n = idx_i16.size
    w = idx_i16.reshape(n // 16, 16).T
    return np.tile(w, (8, 1))


def _chunk_weights(wt):
    """W.T [din, dout] -> [128, (din/128)*dout] SBUF chunk layout."""
    din, dout = wt.shape
    nk = din // P
    return np.ascontiguousarray(wt.reshape(nk, P, dout).transpose(1, 0, 2).reshape(P, nk * dout))


def _chunk_bias(b):
    """[dout] -> [128, ceil(dout/128)]."""
    dout = b.size
    nch = -(-dout // P)
    buf = np.zeros(nch * P, np.float32)
    buf[:dout] = b
    return np.ascontiguousarray(buf.reshape(nch, P).T)


def _host_prep(x_indices):
    """Per-core segment packing. Returns caps (shared) + per-core idx/bagid arrays."""
    idx = np.asarray(x_indices).astype(np.int64)  # [T, B, L]
    per_core = []
    for c in range(N_CORES):
        core_segs = []
        idx_c = idx[:, c * B_CORE:(c + 1) * B_CORE, :]        # [T, 512, L]
        win = idx_c // W_ROWS
        for bt in range(N_BT):
            sub = idx_c[:, bt * 128:(bt + 1) * 128, :]        # [T, 128, L]
            wsub = win[:, bt * 128:(bt + 1) * 128, :]
            for t in range(T):
                for w in range(N_WIN):
                    bags, ls = np.nonzero(wsub[t] == w)
                    li = (sub[t][bags, ls] - w * W_ROWS).astype(np.int16)
                    core_segs.append((li, bags.astype(np.float32)))
        per_core.append(core_segs)
    n_segs = len(per_core[0])
    caps = []
    for s in range(n_segs):
        m = max(per_core[c][s][0].size for c in range(N_CORES))
        caps.append(max(128, -(-m // 128) * 128))
    assert max(caps) <= 8192
    tot16 = sum(cp // 16 for cp in caps)
    gtot = sum(cp // 128 for cp in caps)
    idx_mats, bag_mats = [], []
    for c in range(N_CORES):
        im = np.zeros((P, tot16), np.int16)
        bm = np.full((P, gtot), 255.0, np.float32)
        o16 = 0
        og = 0
        for s, cp in enumerate(caps):
            li, bags = per_core[c][s]
            buf = np.zeros(cp, np.int16)
            buf[:li.size] = li
            im[:, o16:o16 + cp // 16] = _pack_idx_block(buf)
            bb = np.full(cp, 255.0, np.float32)
            bb[:bags.size] = bags
            bm[:, og:og + cp // 128] = bb.reshape(cp // 128, P).T
            o16 += cp // 16
            og += cp // 128
        idx_mats.append(im)
        bag_mats.append(bm)
    return caps, idx_mats, bag_mats, tot16, gtot


def _build(caps, tot16, gtot):
    nc = bacc.Bacc("TRN2", target_bir_lowering=False, debug=False,
                   enable_asserts=False, num_devices=N_CORES, num_swdge_queues=4)
    dt = mybir.dt.float32
    AF = mybir.ActivationFunctionType

    win_d = [nc.dram_tensor(f"win{t}_{w}", [W_ROWS, E], dt, kind="ExternalInput").ap()
             for t in range(T) for w in range(N_WIN)]
    idx_d = nc.dram_tensor("idxs", [P, tot16], mybir.dt.int16, kind="ExternalInput").ap()
    bag_d = nc.dram_tensor("bagids", [P, gtot], dt, kind="ExternalInput").ap()
    xt_d = nc.dram_tensor("xt", [BOT[0], B_CORE], dt, kind="ExternalInput").ap()
    # weights pre-chunked on host: [128, nk*dout]; biases [128, nch]
    wdims = [(BOT[0], BOT[1]), (BOT[1], BOT[2]), (BOT[2], BOT[3]),
             (ZF, TOP[1]), (TOP[1], TOP[2]), (TOP[2], TOP[3])]
    w_d = [nc.dram_tensor(f"w{i}", [P, (din // P) * dout], dt, kind="ExternalInput").ap()
           for i, (din, dout) in enumerate(wdims)]
    b_d = [nc.dram_tensor(f"b{i}", [P, -(-dout // P)], dt, kind="ExternalInput").ap()
           for i, (_, dout) in enumerate(wdims)]
    out_d = nc.dram_tensor("y", [1, B_CORE], dt, kind="ExternalOutput").ap()

    with tile.TileContext(nc) as tc:
        with tc.tile_pool(name="const", bufs=1) as cpool, \
             tc.tile_pool(name="zp", bufs=1) as zp, \
             tc.tile_pool(name="stg", bufs=4) as stg, \
             tc.tile_pool(name="gp", bufs=6) as gp, \
             tc.tile_pool(name="sp", bufs=6) as sp, \
             tc.tile_pool(name="act", bufs=1) as actp, \
             tc.tile_pool(name="pps", bufs=2, space="PSUM") as pps, \
             tc.tile_pool(name="mps", bufs=2, space="PSUM") as mps:

            iota = cpool.tile([P, P], dt)
            nc.gpsimd.iota(iota[:], pattern=[[1, P]], base=0, channel_multiplier=0,
                           allow_small_or_imprecise_dtypes=True)

            bagid = cpool.tile([P, gtot], dt)
            nc.sync.dma_start(out=bagid[:], in_=bag_d[:])

            zt = zp.tile([P, NZCH * B_CORE], dt)
            nc.vector.memset(zt[:], 0.0)

            def load(name, ap_dram, shape):
                t_ = cpool.tile(shape, dt, tag=name)
                nc.sync.dma_start(out=t_[:], in_=ap_dram)
                return t_

            xt = [load(f"xt{k}", xt_d[k * P:(k + 1) * P, :], [P, B_CORE])
                  for k in range(BOT[0] // P)]
            wts = [load(f"w{i}", w_d[i][:, :], [P, (din // P) * dout])
                   for i, (din, dout) in enumerate(wdims)]
            bts = [load(f"b{i}", b_d[i][:, :], [P, -(-dout // P)])
                   for i, (_, dout) in enumerate(wdims)]

            def mlp_layer(src_aps, li, func, out_tag):
                din, dout = wdims[li]
                nk = din // P
                outs = []
                for m in range(-(-dout // P)):
                    mm = min(P, dout - m * P)
                    ps = mps.tile([P, B_CORE], dt, space="PSUM", tag="mlp")
                    for k in range(nk):
                        nc.tensor.matmul(
                            out=ps[:mm, :],
                            lhsT=wts[li][:, k * dout + m * P: k * dout + m * P + mm],
                            rhs=src_aps[k],
                            start=(k == 0), stop=(k == nk - 1))
                    o = actp.tile([P, B_CORE], dt, tag=f"{out_tag}{m}")
                    nc.scalar.activation(out=o[:mm, :], in_=ps[:mm, :], func=func,
                                         bias=bts[li][0:mm, m:m + 1])
                    outs.append(o)
                return outs

            # bottom MLP (feature-major h.T tiles [128, 512])
            h1 = mlp_layer([t_[:, :] for t_ in xt], 0, AF.Relu, "h1")
            h2 = mlp_layer([t_[:, :] for t_ in h1], 1, AF.Relu, "h2")
            h3 = mlp_layer([t_[:, :] for t_ in h2], 2, AF.Relu, "h3")
            nc.vector.tensor_copy(out=zt[0:64, 0:B_CORE], in_=h3[0][0:64, :])

            # embedding gather + selection-matmul pooling
            o16 = 0
            og = 0
            si = 0
            for bt in range(N_BT):
                for t in range(T):
                    fbase = 64 + 64 * t
                    ch = fbase // P
                    prow = fbase % P
                    pooled = pps.tile([P, P], dt, space="PSUM", tag="pooled")
                    n_seg_mm = sum(caps[si + w] // 128 for w in range(N_WIN))
                    mm_i = 0
                    for w in range(N_WIN):
                        cp = caps[si + w]
                        c16 = cp // 16
                        gn = cp // 128
                        stage = stg.tile([P, c16], mybir.dt.int16, tag="stage")
                        nc.sync.dma_start(out=stage[:], in_=idx_d[:, o16:o16 + c16])
                        idx_t = stg.tile([P, c16], mybir.dt.int16, tag="idxt")
                        nc.vector.tensor_copy(out=idx_t[:], in_=stage[:])
                        dst = gp.tile([P, gn, E], dt, tag="dst")
                        nc.gpsimd.dma_gather(
                            out_ap=dst[:], in_ap=win_d[t * N_WIN + w][:],
                            idxs_ap=idx_t[:], num_idxs=cp, num_idxs_reg=cp,
                            elem_size=E, single_packet=False,
                            queue_num=(si + w) % 4)
                        sel = sp.tile([P, gn, P], dt, tag="sel")
                        nc.vector.tensor_tensor(
                            out=sel[:],
                            in0=bagid[:, og:og + gn, None].to_broadcast([P, gn, P]),
                            in1=iota[:, None, :].to_broadcast([P, gn, P]),
                            op=mybir.AluOpType.is_equal)
                        for g in range(gn):
                            nc.tensor.matmul(
                                out=pooled[prow:prow + 64, :],
                                lhsT=dst[:, g, :], rhs=sel[:, g, :],
                                start=(mm_i == 0), stop=(mm_i == n_seg_mm - 1))
                            mm_i += 1
                        o16 += c16
                        og += gn
                    si += N_WIN
                    nc.vector.tensor_copy(
                        out=zt[prow:prow + 64,
                               ch * B_CORE + bt * P: ch * B_CORE + bt * P + P],
                        in_=pooled[prow:prow + 64, :])

            # top MLP
            zsrc = [zt[:, c * B_CORE:(c + 1) * B_CORE] for c in range(NZCH)]
            y1 = mlp_layer(zsrc, 3, AF.Relu, "y1")
            y2 = mlp_layer([t_[:, :] for t_ in y1], 4, AF.Relu, "y2")
            ps = mps.tile([P, B_CORE], dt, space="PSUM", tag="mlp")
            nk = TOP[2] // P
            for k in range(nk):
                nc.tensor.matmul(out=ps[:1, :],
                                 lhsT=wts[5][:, k * TOP[3]: k * TOP[3] + 1],
                                 rhs=y2[k][:, :],
                                 start=(k == 0), stop=(k == nk - 1))
            yo = actp.tile([1, B_CORE], dt, tag="yo")
            nc.scalar.activation(out=yo[:], in_=ps[:1, :], func=AF.Sigmoid,
                                 bias=bts[5][0:1, 0:1])
            nc.sync.dma_start(out=out_d[:], in_=yo[:])

    nc.compile()
    return nc


def kernel(**inputs):
    x_dense = np.asarray(inputs["x_dense"], np.float32)
    x_indices = np.asarray(inputs["x_indices"])
    emb = np.ascontiguousarray(np.asarray(inputs["emb_tables"], np.float32))

    caps, idx_mats, bag_mats, tot16, gtot = _host_prep(x_indices)
    nc = _build(caps, tot16, gtot)

    common = {}
    for t in range(T):
        for w in range(N_WIN):
            common[f"win{t}_{w}"] = np.ascontiguousarray(
                emb[t, w * W_ROWS:(w + 1) * W_ROWS, :])
    w0 = np.asarray(inputs["top_w0"], np.float32)          # [512, 1728]
    w0p = np.zeros((TOP[1], ZF), np.float32)
    w0p[:, :TOP[0]] = w0
    wlist = [np.asarray(inputs["bot_w0"], np.float32).T,
             np.asarray(inputs["bot_w1"], np.float32).T,
             np.asarray(inputs["bot_w2"], np.float32).T,
             w0p.T,
             np.asarray(inputs["top_w1"], np.float32).T,
             np.asarray(inputs["top_w2"], np.float32).T]
    blist = [np.asarray(inputs["bot_b0"], np.float32),
             np.asarray(inputs["bot_b1"], np.float32),
             np.asarray(inputs["bot_b2"], np.float32),
             np.asarray(inputs["top_b0"], np.float32),
             np.asarray(inputs["top_b1"], np.float32),
             np.asarray(inputs["top_b2"], np.float32)]
    for i in range(6):
        common[f"w{i}"] = _chunk_weights(wlist[i])
        common[f"b{i}"] = _chunk_bias(blist[i])

    in_maps = []
    for c in range(N_CORES):
        m = dict(common)
        m["idxs"] = idx_mats[c]
        m["bagids"] = bag_mats[c]
        m["xt"] = np.ascontiguousarray(x_dense[c * B_CORE:(c + 1) * B_CORE, :].T)
        in_maps.append(m)

    res = run_bass_kernel_spmd(nc, in_maps, core_ids=list(range(N_CORES)))
    y = np.empty((B, 1), np.float32)
    for c in range(N_CORES):
        y[c * B_CORE:(c + 1) * B_CORE, 0] = res.results[c]["y"][0]
    return y



# revision 12
# speedup vs baseline: 84.4449x; 84.4449x over previous
"""DLRM (multi-table EmbeddingBag + MLPs) on 8 Trainium2 NeuronCores.

Strategy: data-parallel over batch (512 bags/core); embedding tables
replicated in each core's HBM as bf16, two 25000-row windows pair-packed
per 128-col stripe (row r = [winA[r] | winB[r]], 256 B — the dma_gather
minimum). All rows fetched on-device via dma_gather (4 SWDGE queues).
Pooling: lookups are bag-sorted per segment, so each 128-slot group spans
only a narrow bag range; a rebased one-hot of width W (host-computed max
span across cores, typically ~24-40 instead of 128) is built on DVE and
matmul'd in bf16 into a pre-zeroed PSUM accumulator at per-group column
offsets. MLPs run fused in bf16; PSUM->SBUF eviction uses the Activation
engine.
"""
import numpy as np
import ml_dtypes

import concourse.bacc as bacc
import concourse.bass as bass
import concourse.mybir as mybir
import concourse.tile as tile
from concourse.ap import AP
from concourse.bass_utils import run_bass_kernel_spmd

BF16 = ml_dtypes.bfloat16

T = 26          # tables
R = 100000      # rows per table
E = 64          # embedding dim
B = 4096        # batch
L = 32          # lookups per bag
BOT = [256, 512, 256, 64]
TOP = [E * (1 + T), 512, 256, 1]   # 1728 -> 512 -> 256 -> 1
N_CORES = 8
B_CORE = B // N_CORES               # 512 bags per core
N_BT = B_CORE // 128                # 4 bag-tiles per core
W_ROWS = 25000                      # window rows (<= int16 range)
N_WIN = R // W_ROWS                 # 4 windows per table
N_W = T * N_WIN                     # 104 windows
P = 128
ZF = TOP[0] + 64                    # 1792: zero-padded feature dim
NZCH = ZF // P                      # 14 z chunks
WINS_ROWS = N_W * W_ROWS            # bf16 table rows (128-col padded windows)
PAD_BAG = 224.0                     # rebased bagid for pad slots (never < W)
GP_BUFS = 6                         # dst ring depth


def _pack_idx_block(idx_i16):
    """[n] int16 (n % 16 == 0) -> [128, n//16]: j -> (j%16, j//16), replicated x8."""
    n = idx_i16.size
    w = idx_i16.reshape(n // 16, 16).T
    return np.tile(w, (8, 1))


def _chunk_weights(wt):
    """W.T [din, dout] -> [128, (din/128)*dout] SBUF chunk layout (bf16)."""
    din, dout = wt.shape
    nk = din // P
    return np.ascontiguousarray(
        wt.reshape(nk, P, dout).transpose(1, 0, 2).reshape(P, nk * dout)
    ).astype(BF16)


def _chunk_bias(b):
    """[dout] -> [128, ceil(dout/128)] fp32."""
    dout = b.size
    nch = -(-dout // P)
    buf = np.zeros(nch * P, np.float32)
    buf[:dout] = b
    return np.ascontiguousarray(buf.reshape(nch, P).T)


def _host_prep(x_indices):
    """Per-core segment packing with bag-sorted slots + narrow rebased one-hots.

    Returns (segmeta, idx_mats, bag_mats, tot16, gtot):
      segmeta: per segment dict(cap, gn, W, fb[gn]) shared by all cores
      idx_mats: per-core [128, tot16] int16 window-local indices
      bag_mats: per-core [128, gtot*Wmax-packed]  -- actually [128, gtot] rebased
    """
    idx = np.asarray(x_indices).astype(np.int64)  # [T, B, L]
    per_core = []
    for c in range(N_CORES):
        core_segs = []
        idx_c = idx[:, c * B_CORE:(c + 1) * B_CORE, :]        # [T, 512, L]
        win = idx_c // W_ROWS
        for bt in range(N_BT):
            sub = idx_c[:, bt * 128:(bt + 1) * 128, :]        # [T, 128, L]
            wsub = win[:, bt * 128:(bt + 1) * 128, :]
            for t in range(T):
                for w in range(N_WIN):
                    bags, ls = np.nonzero(wsub[t] == w)       # bags ascending
                    li = (sub[t][bags, ls] - w * W_ROWS).astype(np.int16)
                    core_segs.append((li, bags.astype(np.int64)))
        per_core.append(core_segs)
    n_segs = len(per_core[0])

    caps0 = []
    for s in range(n_segs):
        m = max(per_core[c][s][0].size for c in range(N_CORES))
        caps0.append(max(16, -(-m // 16) * 16))
    gn_max = max(-(-cp // 128) for cp in caps0)
    # First GP_BUFS segments gather the full dst ring buffer so later
    # partially-written uses only ever see finite stale bf16 (never uninit).
    for s in range(min(GP_BUFS, n_segs)):
        caps0[s] = gn_max * 128

    segmeta = []
    for s in range(n_segs):
        cap = caps0[s]
        gn = -(-cap // 128)
        fb = []
        ww = 1
        for g in range(gn):
            lo, hi = None, None
            for c in range(N_CORES):
                bags = per_core[c][s][1]
                seg = bags[g * 128:(g + 1) * 128]
                if seg.size:
                    lo = seg[0] if lo is None else min(lo, seg[0])
                    hi = seg[-1] if hi is None else max(hi, seg[-1])
            if lo is None:
                fb.append(0)
            else:
                fb.append(int(lo))
                ww = max(ww, int(hi - lo + 1))
        ww = min(P, -(-ww // 4) * 4)
        fb = [min(f, P - ww) for f in fb]
        segmeta.append({"cap": cap, "gn": gn, "W": ww, "fb": fb})
    assert max(sm["cap"] for sm in segmeta) <= 8192
    tot16 = sum(sm["cap"] // 16 for sm in segmeta)
    gtot = sum(sm["gn"] for sm in segmeta)

    idx_mats, bag_mats = [], []
    for c in range(N_CORES):
        im = np.zeros((P, tot16), np.int16)
        bm = np.full((P, gtot), PAD_BAG, np.float32)
        o16 = 0
        og = 0
        for s, sm in enumerate(segmeta):
            cap, gn, fb = sm["cap"], sm["gn"], sm["fb"]
            li, bags = per_core[c][s]
            buf = np.zeros(cap, np.int16)
            buf[:li.size] = li
            im[:, o16:o16 + cap // 16] = _pack_idx_block(buf)
            bb = np.full(gn * 128, PAD_BAG, np.float32)
            rb = bags.astype(np.float32).copy()
            for g in range(gn):
                a, b = g * 128, min((g + 1) * 128, rb.size)
                if a < rb.size:
                    rb[a:b] -= fb[g]
            bb[:rb.size] = rb
            bm[:, og:og + gn] = bb.reshape(gn, P).T
            o16 += cap // 16
            og += gn
        idx_mats.append(im)
        bag_mats.append(bm.astype(BF16))
    return segmeta, idx_mats, bag_mats, tot16, gtot


def _pack_wins(emb):
    """emb [T, R, E] fp32 -> bf16 [WINS_ROWS, 128], one window per 25000-row
    stripe, embedding in cols 0:64 (gather bases must be 256-B aligned)."""
    eb = np.asarray(emb, np.float32).astype(BF16)
    eb = eb.reshape(N_W, W_ROWS, E)
    wins = np.zeros((WINS_ROWS, 2 * E), BF16)
    wins.reshape(N_W, W_ROWS, 2 * E)[:, :, :E] = eb
    return wins


def _queue_phase_ok(nc):
    """Tile assigns SWDGE completion sems by rotating 8 lanes over Pool-DMA
    instructions in final program order; a sem lane may only be incremented
    from one queue. Verify our static queue_num assignment matches."""
    def walk(block, out):
        for inst in block.instructions:
            out.append(inst)
            for b in getattr(inst, 'blocks', []) or []:
                walk(b, out)
    insts = []
    for b in nc.m.functions[0].blocks:
        walk(b, insts)
    lane = 0
    lock = {}
    for i in insts:
        if isinstance(i, mybir.InstDMAGatherAnt):
            q = i.queue_num
            if lock.setdefault(lane, q) != q:
                return False
            lane = (lane + 1) % 8
    return True


def _build(segmeta, tot16, gtot, n_queues=4):
    GN_MAX = max(sm["gn"] for sm in segmeta)
    nc = bacc.Bacc("TRN2", target_bir_lowering=False, debug=False,
                   enable_asserts=False, num_devices=N_CORES,
                   num_swdge_queues=n_queues)
    f32 = mybir.dt.float32
    bf16 = mybir.dt.bfloat16
    AF = mybir.ActivationFunctionType

    wins_d = nc.dram_tensor("wins", [WINS_ROWS, 2 * E], bf16,
                            kind="ExternalInput").ap()
    idx_d = nc.dram_tensor("idxs", [P, tot16], mybir.dt.int16,
                           kind="ExternalInput").ap()
    bag_d = nc.dram_tensor("bagids", [P, gtot], bf16, kind="ExternalInput").ap()
    xt_d = nc.dram_tensor("xt", [BOT[0], B_CORE], bf16, kind="ExternalInput").ap()
    wdims = [(BOT[0], BOT[1]), (BOT[1], BOT[2]), (BOT[2], BOT[3]),
             (ZF, TOP[1]), (TOP[1], TOP[2]), (TOP[2], TOP[3])]
    w_d = [nc.dram_tensor(f"w{i}", [P, (din // P) * dout], bf16,
                          kind="ExternalInput").ap()
           for i, (din, dout) in enumerate(wdims)]
    b_d = [nc.dram_tensor(f"b{i}", [P, -(-dout // P)], f32,
                          kind="ExternalInput").ap()
           for i, (_, dout) in enumerate(wdims)]
    out_d = nc.dram_tensor("y", [1, B_CORE], f32, kind="ExternalOutput").ap()

    def win_view(j):
        """AP over window j: [W_ROWS, 128] bf16 (256-B-aligned base)."""
        base = j * W_ROWS * (2 * E)
        return AP(tensor=wins_d.tensor, offset=base,
                  ap=[[2 * E, W_ROWS], [1, 2 * E]])

    with tile.TileContext(nc) as tc:
        with tc.tile_pool(name="const", bufs=1) as cpool, \
             tc.tile_pool(name="zp", bufs=1) as zp, \
             tc.tile_pool(name="stg", bufs=4) as stg, \
             tc.tile_pool(name="gp", bufs=GP_BUFS) as gp, \
             tc.tile_pool(name="sp", bufs=6) as sp, \
             tc.tile_pool(name="act", bufs=1) as actp, \
             tc.tile_pool(name="pps", bufs=2, space="PSUM") as pps, \
             tc.tile_pool(name="mps", bufs=2, space="PSUM") as mps:

            iota = cpool.tile([P, P], bf16)
            nc.gpsimd.iota(iota[:], pattern=[[1, P]], base=0, channel_multiplier=0,
                           allow_small_or_imprecise_dtypes=True)
            zlhs = cpool.tile([P, 64], bf16)
            nc.vector.memset(zlhs[:], 0.0)

            bagid = cpool.tile([P, gtot], bf16)
            nc.sync.dma_start(out=bagid[:], in_=bag_d[:])

            zt = zp.tile([P, NZCH * B_CORE], bf16)
            nc.vector.memset(zt[:], 0.0)

            def load(name, ap_dram, shape, dtype):
                t_ = cpool.tile(shape, dtype, tag=name)
                nc.sync.dma_start(out=t_[:], in_=ap_dram)
                return t_

            xt = [load(f"xt{k}", xt_d[k * P:(k + 1) * P, :], [P, B_CORE], bf16)
                  for k in range(BOT[0] // P)]
            wts = [load(f"w{i}", w_d[i][:, :], [P, (din // P) * dout], bf16)
                   for i, (din, dout) in enumerate(wdims)]
            bts = [load(f"b{i}", b_d[i][:, :], [P, -(-dout // P)], f32)
                   for i, (_, dout) in enumerate(wdims)]

            def mlp_layer(src_aps, li, func, out_tag):
                din, dout = wdims[li]
                nk = din // P
                outs = []
                for m in range(-(-dout // P)):
                    mm = min(P, dout - m * P)
                    ps = mps.tile([P, B_CORE], f32, space="PSUM", tag="mlp")
                    for k in range(nk):
                        nc.tensor.matmul(
                            out=ps[:mm, :],
                            lhsT=wts[li][:, k * dout + m * P: k * dout + m * P + mm],
                            rhs=src_aps[k],
                            start=(k == 0), stop=(k == nk - 1))
                    o = actp.tile([P, B_CORE], bf16, tag=f"{out_tag}{m}")
                    nc.scalar.activation(out=o[:mm, :], in_=ps[:mm, :], func=func,
                                         bias=bts[li][0:mm, m:m + 1])
                    outs.append(o)
                return outs

            # bottom MLP (feature-major h.T tiles [128, 512])
            h1 = mlp_layer([t_[:, :] for t_ in xt], 0, AF.Relu, "h1")
            h2 = mlp_layer([t_[:, :] for t_ in h1], 1, AF.Relu, "h2")
            h3 = mlp_layer([t_[:, :] for t_ in h2], 2, AF.Relu, "h3")
            nc.vector.tensor_copy(out=zt[0:64, 0:B_CORE], in_=h3[0][0:64, :])

            # embedding gather + narrow selection-matmul pooling
            o16 = 0
            og = 0
            si = 0
            for bt in range(N_BT):
                for t in range(T):
                    fbase = 64 + 64 * t
                    ch = fbase // P
                    prow = fbase % P
                    pooled = pps.tile([P, P], f32, space="PSUM", tag="pooled")
                    # PSUM pre-zero: zero lhsT x anything, full 128 cols
                    nc.tensor.matmul(out=pooled[prow:prow + 64, :],
                                     lhsT=zlhs[:, :], rhs=iota[:, :],
                                     start=True, stop=False,
                                     skip_group_check=True)
                    seg4 = [segmeta[si + w] for w in range(N_WIN)]
                    n_seg_mm = sum(sm["gn"] for sm in seg4)
                    c16_4 = sum(sm["cap"] // 16 for sm in seg4)
                    stage = stg.tile([P, c16_4], mybir.dt.int16, tag="stage")
                    nc.sync.dma_start(out=stage[:], in_=idx_d[:, o16:o16 + c16_4])
                    o16 += c16_4
                    mm_i = 0
                    so16 = 0
                    for w in range(N_WIN):
                        sm = seg4[w]
                        cp, gn, ww, fb = sm["cap"], sm["gn"], sm["W"], sm["fb"]
                        c16 = cp // 16
                        dstf = gp.tile([P, GN_MAX, 2 * E], bf16, tag="dst")
                        dst = dstf[:, 0:gn, :]
                        nc.gpsimd.dma_gather(
                            out_ap=dst, in_ap=win_view(t * N_WIN + w),
                            idxs_ap=stage[:, so16:so16 + c16],
                            num_idxs=cp, num_idxs_reg=cp,
                            elem_size=2 * E, single_packet=False,
                            queue_num=(si + w) % n_queues)
                        so16 += c16
                        sel = sp.tile([P, gn, ww], bf16, tag="sel")
                        nc.vector.tensor_tensor(
                            out=sel[:],
                            in0=bagid[:, og:og + gn, None].to_broadcast([P, gn, ww]),
                            in1=iota[:, None, 0:ww].to_broadcast([P, gn, ww]),
                            op=mybir.AluOpType.is_equal)
                        for g in range(gn):
                            nc.tensor.matmul(
                                out=pooled[prow:prow + 64, fb[g]:fb[g] + ww],
                                lhsT=dstf[:, g, 0:E], rhs=sel[:, g, :],
                                start=False, stop=(mm_i == n_seg_mm - 1),
                                skip_group_check=True)
                            mm_i += 1
                        og += gn
                    si += N_WIN
                    nc.scalar.activation(
                        out=zt[prow:prow + 64,
                               ch * B_CORE + bt * P: ch * B_CORE + bt * P + P],
                        in_=pooled[prow:prow + 64, :], func=AF.Copy)

            # top MLP
            zsrc = [zt[:, c * B_CORE:(c + 1) * B_CORE] for c in range(NZCH)]
            y1 = mlp_layer(zsrc, 3, AF.Relu, "y1")
            y2 = mlp_layer([t_[:, :] for t_ in y1], 4, AF.Relu, "y2")
            ps = mps.tile([P, B_CORE], f32, space="PSUM", tag="mlp")
            nk = TOP[2] // P
            for k in range(nk):
                nc.tensor.matmul(out=ps[:1, :],
                                 lhsT=wts[5][:, k * TOP[3]: k * TOP[3] + 1],
                                 rhs=y2[k][:, :],
                                 start=(k == 0), stop=(k == nk - 1))
            yo = actp.tile([1, B_CORE], f32, tag="yo")
            nc.scalar.activation(out=yo[:], in_=ps[:1, :], func=AF.Sigmoid,
                                 bias=bts[5][0:1, 0:1])
            nc.sync.dma_start(out=out_d[:], in_=yo[:])

    nc.compile()
    if n_queues > 1 and not _queue_phase_ok(nc):
        return _build(segmeta, tot16, gtot, n_queues=1)
    return nc


def _make_common(inputs):
    """Input tensors shared by all cores."""
    common = {"wins": _pack_wins(inputs["emb_tables"])}
    w0 = np.asarray(inputs["top_w0"], np.float32)          # [512, 1728]
    w0p = np.zeros((TOP[1], ZF), np.float32)
    w0p[:, :TOP[0]] = w0
    wlist = [np.asarray(inputs["bot_w0"], np.float32).T,
             np.asarray(inputs["bot_w1"], np.float32).T,
             np.asarray(inputs["bot_w2"], np.float32).T,
             w0p.T,
             np.asarray(inputs["top_w1"], np.float32).T,
             np.asarray(inputs["top_w2"], np.float32).T]
    blist = [np.asarray(inputs["bot_b0"], np.float32),
             np.asarray(inputs["bot_b1"], np.float32),
             np.asarray(inputs["bot_b2"], np.float32),
             np.asarray(inputs["top_b0"], np.float32),
             np.asarray(inputs["top_b1"], np.float32),
             np.asarray(inputs["top_b2"], np.float32)]
    for i in range(6):
        common[f"w{i}"] = _chunk_weights(wlist[i])
        common[f"b{i}"] = _chunk_bias(blist[i])
    return common


def _make_in_maps(inputs, idx_mats, bag_mats):
    common = _make_common(inputs)
    x_dense = np.asarray(inputs["x_dense"], np.float32)
    in_maps = []
    for c in range(N_CORES):
        m = dict(common)
        m["idxs"] = idx_mats[c]
        m["bagids"] = bag_mats[c]
        m["xt"] = np.ascontiguousarray(
            x_dense[c * B_CORE:(c + 1) * B_CORE, :].T).astype(BF16)
        in_maps.append(m)
    return in_maps


def kernel(**inputs):
    segmeta, idx_mats, bag_mats, tot16, gtot = _host_prep(inputs["x_indices"])
    nc = _build(segmeta, tot16, gtot)
    in_maps = _make_in_maps(inputs, idx_mats, bag_mats)
    res = run_bass_kernel_spmd(nc, in_maps, core_ids=list(range(N_CORES)))
    y = np.empty((B, 1), np.float32)
    for c in range(N_CORES):
        y[c * B_CORE:(c + 1) * B_CORE, 0] = res.results[c]["y"][0]
    return y
